# revision 9
# baseline (speedup 1.0000x reference)
"""BinaryLinear (LLaMA-7B up-projection with sign-binarized weights) on 8 TRN2
NeuronCores.

Computes out = x @ sign(weight).T + bias for
  x      [4, 2048, 4096] fp16
  weight [11008, 4096]   fp16
  bias   [11008]         fp16
-> out   [4, 2048, 11008] fp16

Sharding: 2D tensor-parallel — features split in 2 halves of 5504 (= 43 tiles
of 128), tokens split in 4 quarters of 2048. Core c handles feature half
c % 2, token quarter c // 2. No collectives; the host gathers the 8 disjoint
output shards.

Per-core device kernel:
  - The x shard, pre-transposed to [128ki, 32ko, 2048t] on the host, is
    DMA'd once and stays SBUF-resident (16.8MB).
  - sign(weight).T streams through SBUF in 43 feature chunks [128ki, 32ko,
    128f] (1.4MB each, host pre-swizzled so each chunk is one contiguous
    per-partition run), triple-buffered.
  - For each (feature tile f, token block t of 512): 32 accumulating matmuls
    (lhsT = weight tile [128k, 128f], rhs = x tile [128k, 512t]) into one
    PSUM bank; ScalarE applies per-partition bias + fp32->fp16 cast into a
    [128, 2048] staging row; one DMA per feature tile writes it out.

sign(weight), the layout swizzles, and the output gather run on the host —
layout prep, off the device critical path.
"""

import numpy as np

B, S, IN, OUT = 4, 2048, 4096, 11008
TOKENS = B * S  # 8192
NCORES = 8
FSPLIT = 2  # feature halves
TSPLIT = 4  # token quarters
F_SHARD = OUT // FSPLIT  # 5504
T_SHARD = TOKENS // TSPLIT  # 2048
P = 128
F_TILES = F_SHARD // P  # 43
K_TILES = IN // P  # 32
NB = 512  # tokens per PSUM epoch
T_BLOCKS = T_SHARD // NB  # 4

_cached_nc = None


def _build_nc():
    import concourse.mybir as mybir
    import concourse.tile as tile
    from concourse import bacc

    nc = bacc.Bacc(
        "TRN2",
        target_bir_lowering=False,
        debug=False,
        enable_asserts=False,
    )

    xt = nc.dram_tensor(
        "xt", [P, K_TILES, T_SHARD], mybir.dt.float16, kind="ExternalInput"
    )
    wt = nc.dram_tensor(
        "wt", [F_TILES, P, K_TILES, P], mybir.dt.float16, kind="ExternalInput"
    )
    bias = nc.dram_tensor("bias", [P, F_TILES], mybir.dt.float32, kind="ExternalInput")
    out = nc.dram_tensor(
        "out", [F_SHARD, T_SHARD], mybir.dt.float16, kind="ExternalOutput"
    )

    xt_ap = xt.ap()
    wt_ap = wt.ap()
    out_ap = out.ap()

    # First WARM_F feature tiles run k-outer across all 8 PSUM banks so the
    # PE starts as soon as the first per-ko x chunk lands, overlapping the
    # 16.8MB x-shard load instead of idling behind it.
    WARM_F = 2

    with tile.TileContext(nc) as tc:
        with (
            tc.tile_pool(name="x", bufs=1) as xp,
            tc.tile_pool(name="const", bufs=1) as cp,
            tc.tile_pool(name="wwarm", bufs=1) as wwp,
            tc.tile_pool(name="w", bufs=3) as wp,
            tc.tile_pool(name="o", bufs=3) as op,
            tc.tile_pool(name="ps", bufs=8, space="PSUM") as pp,
        ):
            # Warm-phase weights split into per-8ko tiles so the very first
            # matmuls are gated on ~350KB, not a full 1.4MB chunk.
            WKC = 8  # ko per warm w chunk
            w_warm = [[None] * (K_TILES // WKC) for _ in range(WARM_F)]
            for f in range(WARM_F):
                for c in range(1):  # chunk 0 of each warm tile loads first
                    w_sb = wwp.tile(
                        [P, WKC, P], mybir.dt.float16, name=f"ww{f}c{c}", tag=f"ww{f}c{c}"
                    )
                    nc.sync.dma_start(w_sb[:], wt_ap[f, :, c * WKC : (c + 1) * WKC, :])
                    w_warm[f][c] = w_sb

            # x shard as 32 per-ko tiles so deps are chunk-granular.
            xs = []
            for k in range(K_TILES):
                x_sb = xp.tile([P, T_SHARD], mybir.dt.float16, tag=f"x{k}")
                nc.sync.dma_start(x_sb[:], xt_ap[:, k, :])
                xs.append(x_sb)
            bias_sb = cp.tile([P, F_TILES], mybir.dt.float32)
            nc.sync.dma_start(bias_sb[:], bias.ap()[:])
            # Remaining warm w chunks (needed only from k=8 on).
            for c in range(1, K_TILES // WKC):
                for f in range(WARM_F):
                    w_sb = wwp.tile(
                        [P, WKC, P], mybir.dt.float16, name=f"ww{f}c{c}", tag=f"ww{f}c{c}"
                    )
                    nc.sync.dma_start(w_sb[:], wt_ap[f, :, c * WKC : (c + 1) * WKC, :])
                    w_warm[f][c] = w_sb

            # Phase A: k-outer warm start for f = 0..WARM_F-1.
            ps_warm = [
                [
                    pp.tile([P, NB], mybir.dt.float32, name="ps", tag="ps")
                    for _ in range(T_BLOCKS)
                ]
                for _ in range(WARM_F)
            ]
            for k in range(K_TILES):
                for f in range(WARM_F):
                    for t in range(T_BLOCKS):
                        nc.tensor.matmul(
                            ps_warm[f][t][:],
                            w_warm[f][k // WKC][:, k % WKC, :],
                            xs[k][:, t * NB : (t + 1) * NB],
                            start=(k == 0),
                            stop=(k == K_TILES - 1),
                        )
            for f in range(WARM_F):
                o_sb = op.tile([P, T_SHARD], mybir.dt.float16)
                for t in range(T_BLOCKS):
                    nc.scalar.activation(
                        o_sb[:, t * NB : (t + 1) * NB],
                        ps_warm[f][t][:],
                        mybir.ActivationFunctionType.Identity,
                        bias=bias_sb[:, f : f + 1],
                    )
                nc.sync.dma_start(out_ap[f * P : (f + 1) * P, :], o_sb[:])

            # Phase B: f-outer steady state, x fully resident. The last f
            # tile is evicted per token block so the kernel tail is one
            # small DMA, not a 512KB one.
            for f in range(WARM_F, F_TILES):
                w_sb = wp.tile([P, K_TILES, P], mybir.dt.float16, tag="w")
                nc.sync.dma_start(w_sb[:], wt_ap[f])
                last = f == F_TILES - 1
                o_sb = op.tile([P, T_SHARD], mybir.dt.float16)
                for t in range(T_BLOCKS):
                    ps = pp.tile([P, NB], mybir.dt.float32)
                    for k in range(K_TILES):
                        nc.tensor.matmul(
                            ps[:],
                            w_sb[:, k, :],
                            xs[k][:, t * NB : (t + 1) * NB],
                            start=(k == 0),
                            stop=(k == K_TILES - 1),
                        )
                    nc.scalar.activation(
                        o_sb[:, t * NB : (t + 1) * NB],
                        ps[:],
                        mybir.ActivationFunctionType.Identity,
                        bias=bias_sb[:, f : f + 1],
                    )
                    if last:
                        nc.sync.dma_start(
                            out_ap[f * P : (f + 1) * P, t * NB : (t + 1) * NB],
                            o_sb[:, t * NB : (t + 1) * NB],
                        )
                if not last:
                    nc.sync.dma_start(out_ap[f * P : (f + 1) * P, :], o_sb[:])
    nc.compile()
    return nc


def _get_nc():
    global _cached_nc
    if _cached_nc is None:
        _cached_nc = _build_nc()
    return _cached_nc


_last_results = None  # BassKernelResults of the most recent run (for test harness)


def kernel(x, weight, bias, _trace=False):
    global _last_results
    from concourse.bass_utils import run_bass_kernel_spmd

    x = np.asarray(x)
    weight = np.asarray(weight)
    bias = np.asarray(bias)
    assert x.shape == (B, S, IN) and weight.shape == (OUT, IN) and bias.shape == (OUT,)

    nc = _get_nc()

    # xT [IN, TOKENS] -> per token-quarter [128ki, 32ko, 2048t]
    xt = x.reshape(TOKENS, IN).T  # [IN, TOKENS] (view)
    xt_quarters = [
        np.ascontiguousarray(
            xt[:, i * T_SHARD : (i + 1) * T_SHARD]
            .reshape(K_TILES, P, T_SHARD)
            .transpose(1, 0, 2)
        )
        for i in range(TSPLIT)
    ]

    ws = np.sign(weight).astype(np.float16)  # [OUT, IN]
    bias_f32 = bias.astype(np.float32)
    # per feature-half: [43ft, 128ki, 32ko, 128f] chunk-major swizzle
    wt_halves = []
    bias_halves = []
    for j in range(FSPLIT):
        wsj = ws[j * F_SHARD : (j + 1) * F_SHARD, :].T  # [IN, F_SHARD] (view)
        wt_halves.append(
            np.ascontiguousarray(
                wsj.reshape(K_TILES, P, F_TILES, P).transpose(2, 1, 0, 3)
            )
        )
        bias_halves.append(
            np.ascontiguousarray(
                bias_f32[j * F_SHARD : (j + 1) * F_SHARD].reshape(F_TILES, P).T
            )
        )

    in_maps = []
    for c in range(NCORES):
        j, i = c % FSPLIT, c // FSPLIT
        in_maps.append(
            {"xt": xt_quarters[i], "wt": wt_halves[j], "bias": bias_halves[j]}
        )

    res = run_bass_kernel_spmd(nc, in_maps, core_ids=list(range(NCORES)), trace=_trace)
    _last_results = res

    full = np.empty((OUT, TOKENS), dtype=np.float16)
    for c in range(NCORES):
        j, i = c % FSPLIT, c // FSPLIT
        full[
            j * F_SHARD : (j + 1) * F_SHARD, i * T_SHARD : (i + 1) * T_SHARD
        ] = res.results[c]["out"]
    return np.ascontiguousarray(full.T).reshape(B, S, OUT)


# revision 10
# speedup vs baseline: 1.0219x; 1.0219x over previous
"""BinaryLinear (LLaMA-7B up-projection with sign-binarized weights) on 8 TRN2
NeuronCores.

Computes out = x @ sign(weight).T + bias for
  x      [4, 2048, 4096] fp16
  weight [11008, 4096]   fp16
  bias   [11008]         fp16
-> out   [4, 2048, 11008] fp16

Sharding: 2D tensor-parallel — features split in 2 halves of 5504 (= 43 tiles
of 128), tokens split in 4 quarters of 2048. Core c handles feature half
c % 2, token quarter c // 2. No collectives; the host gathers the 8 disjoint
output shards.

Per-core device kernel:
  - The x shard, pre-transposed to [128ki, 32ko, 2048t] on the host, is
    DMA'd once and stays SBUF-resident (16.8MB).
  - sign(weight).T streams through SBUF in 43 feature chunks [128ki, 32ko,
    128f] (1.4MB each, host pre-swizzled so each chunk is one contiguous
    per-partition run), triple-buffered.
  - For each (feature tile f, token block t of 512): 32 accumulating matmuls
    (lhsT = weight tile [128k, 128f], rhs = x tile [128k, 512t]) into one
    PSUM bank; ScalarE applies per-partition bias + fp32->fp16 cast into a
    [128, 2048] staging row; one DMA per feature tile writes it out.

sign(weight), the layout swizzles, and the output gather run on the host —
layout prep, off the device critical path.
"""

import numpy as np

B, S, IN, OUT = 4, 2048, 4096, 11008
TOKENS = B * S  # 8192
NCORES = 8
FSPLIT = 2  # feature halves
TSPLIT = 4  # token quarters
F_SHARD = OUT // FSPLIT  # 5504
T_SHARD = TOKENS // TSPLIT  # 2048
P = 128
F_TILES = F_SHARD // P  # 43
K_TILES = IN // P  # 32
NB = 512  # tokens per PSUM epoch
T_BLOCKS = T_SHARD // NB  # 4

_cached_nc = None


def _build_nc():
    import concourse.mybir as mybir
    import concourse.tile as tile
    from concourse import bacc

    nc = bacc.Bacc(
        "TRN2",
        target_bir_lowering=False,
        debug=False,
        enable_asserts=False,
    )

    xt = nc.dram_tensor(
        "xt", [P, K_TILES, T_SHARD], mybir.dt.float16, kind="ExternalInput"
    )
    wt = nc.dram_tensor(
        "wt", [F_TILES, P, K_TILES, P], mybir.dt.float16, kind="ExternalInput"
    )
    bias = nc.dram_tensor("bias", [P, F_TILES], mybir.dt.float32, kind="ExternalInput")
    out = nc.dram_tensor(
        "out", [F_SHARD, T_SHARD], mybir.dt.float16, kind="ExternalOutput"
    )

    xt_ap = xt.ap()
    wt_ap = wt.ap()
    out_ap = out.ap()

    # First WARM_F feature tiles run k-outer across all 8 PSUM banks so the
    # PE starts as soon as the first per-ko x chunk lands, overlapping the
    # 16.8MB x-shard load instead of idling behind it.
    WARM_F = 2

    with tile.TileContext(nc) as tc:
        with (
            tc.tile_pool(name="x", bufs=1) as xp,
            tc.tile_pool(name="const", bufs=1) as cp,
            tc.tile_pool(name="wwarm", bufs=1) as wwp,
            tc.tile_pool(name="w", bufs=3) as wp,
            tc.tile_pool(name="o", bufs=3) as op,
            tc.tile_pool(name="ps", bufs=8, space="PSUM") as pp,
        ):
            # Warm-phase weights split into per-8ko tiles so the very first
            # matmuls are gated on ~350KB, not a full 1.4MB chunk. The
            # chunk-c DMAs are interleaved into the x-chunk DMA stream just
            # ahead of when the k-outer warm loop will need them (queues
            # drain roughly in issue order).
            WKC = 8  # ko per warm w chunk
            NWC = K_TILES // WKC
            w_warm = [[None] * NWC for _ in range(WARM_F)]

            def _emit_warm_w(c):
                for f in range(WARM_F):
                    w_sb = wwp.tile(
                        [P, WKC, P], mybir.dt.float16, name=f"ww{f}c{c}", tag=f"ww{f}c{c}"
                    )
                    nc.sync.dma_start(w_sb[:], wt_ap[f, :, c * WKC : (c + 1) * WKC, :])
                    w_warm[f][c] = w_sb

            # x shard as 32 per-ko tiles so deps are chunk-granular.
            xs = [None] * K_TILES

            def _emit_x(k):
                x_sb = xp.tile([P, T_SHARD], mybir.dt.float16, name=f"x{k}", tag=f"x{k}")
                nc.sync.dma_start(x_sb[:], xt_ap[:, k, :])
                xs[k] = x_sb

            _emit_warm_w(0)
            for k in range(0, 3):
                _emit_x(k)
            _emit_warm_w(1)
            for k in range(3, 8):
                _emit_x(k)
            _emit_warm_w(2)
            for k in range(8, 16):
                _emit_x(k)
            _emit_warm_w(3)
            for k in range(16, K_TILES):
                _emit_x(k)
            bias_sb = cp.tile([P, F_TILES], mybir.dt.float32)
            nc.sync.dma_start(bias_sb[:], bias.ap()[:])

            # Phase A: k-outer warm start for f = 0..WARM_F-1.
            ps_warm = [
                [
                    pp.tile([P, NB], mybir.dt.float32, name="ps", tag="ps")
                    for _ in range(T_BLOCKS)
                ]
                for _ in range(WARM_F)
            ]
            for k in range(K_TILES):
                for f in range(WARM_F):
                    for t in range(T_BLOCKS):
                        nc.tensor.matmul(
                            ps_warm[f][t][:],
                            w_warm[f][k // WKC][:, k % WKC, :],
                            xs[k][:, t * NB : (t + 1) * NB],
                            start=(k == 0),
                            stop=(k == K_TILES - 1),
                        )
            for f in range(WARM_F):
                o_sb = op.tile([P, T_SHARD], mybir.dt.float16)
                for t in range(T_BLOCKS):
                    nc.scalar.activation(
                        o_sb[:, t * NB : (t + 1) * NB],
                        ps_warm[f][t][:],
                        mybir.ActivationFunctionType.Identity,
                        bias=bias_sb[:, f : f + 1],
                    )
                nc.sync.dma_start(out_ap[f * P : (f + 1) * P, :], o_sb[:])

            # Phase B: f-outer steady state, x fully resident. The last f
            # tile is evicted per token block so the kernel tail is one
            # small DMA, not a 512KB one.
            for f in range(WARM_F, F_TILES):
                w_sb = wp.tile([P, K_TILES, P], mybir.dt.float16, tag="w")
                nc.sync.dma_start(w_sb[:], wt_ap[f])
                last = f == F_TILES - 1
                o_sb = op.tile([P, T_SHARD], mybir.dt.float16)
                for t in range(T_BLOCKS):
                    ps = pp.tile([P, NB], mybir.dt.float32)
                    for k in range(K_TILES):
                        nc.tensor.matmul(
                            ps[:],
                            w_sb[:, k, :],
                            xs[k][:, t * NB : (t + 1) * NB],
                            start=(k == 0),
                            stop=(k == K_TILES - 1),
                        )
                    nc.scalar.activation(
                        o_sb[:, t * NB : (t + 1) * NB],
                        ps[:],
                        mybir.ActivationFunctionType.Identity,
                        bias=bias_sb[:, f : f + 1],
                    )
                    if last:
                        nc.sync.dma_start(
                            out_ap[f * P : (f + 1) * P, t * NB : (t + 1) * NB],
                            o_sb[:, t * NB : (t + 1) * NB],
                        )
                if not last:
                    nc.sync.dma_start(out_ap[f * P : (f + 1) * P, :], o_sb[:])
    nc.compile()
    return nc


def _get_nc():
    global _cached_nc
    if _cached_nc is None:
        _cached_nc = _build_nc()
    return _cached_nc


_last_results = None  # BassKernelResults of the most recent run (for test harness)


def kernel(x, weight, bias, _trace=False):
    global _last_results
    from concourse.bass_utils import run_bass_kernel_spmd

    x = np.asarray(x)
    weight = np.asarray(weight)
    bias = np.asarray(bias)
    assert x.shape == (B, S, IN) and weight.shape == (OUT, IN) and bias.shape == (OUT,)

    nc = _get_nc()

    # xT [IN, TOKENS] -> per token-quarter [128ki, 32ko, 2048t]
    xt = x.reshape(TOKENS, IN).T  # [IN, TOKENS] (view)
    xt_quarters = [
        np.ascontiguousarray(
            xt[:, i * T_SHARD : (i + 1) * T_SHARD]
            .reshape(K_TILES, P, T_SHARD)
            .transpose(1, 0, 2)
        )
        for i in range(TSPLIT)
    ]

    ws = np.sign(weight).astype(np.float16)  # [OUT, IN]
    bias_f32 = bias.astype(np.float32)
    # per feature-half: [43ft, 128ki, 32ko, 128f] chunk-major swizzle
    wt_halves = []
    bias_halves = []
    for j in range(FSPLIT):
        wsj = ws[j * F_SHARD : (j + 1) * F_SHARD, :].T  # [IN, F_SHARD] (view)
        wt_halves.append(
            np.ascontiguousarray(
                wsj.reshape(K_TILES, P, F_TILES, P).transpose(2, 1, 0, 3)
            )
        )
        bias_halves.append(
            np.ascontiguousarray(
                bias_f32[j * F_SHARD : (j + 1) * F_SHARD].reshape(F_TILES, P).T
            )
        )

    in_maps = []
    for c in range(NCORES):
        j, i = c % FSPLIT, c // FSPLIT
        in_maps.append(
            {"xt": xt_quarters[i], "wt": wt_halves[j], "bias": bias_halves[j]}
        )

    res = run_bass_kernel_spmd(nc, in_maps, core_ids=list(range(NCORES)), trace=_trace)
    _last_results = res

    full = np.empty((OUT, TOKENS), dtype=np.float16)
    for c in range(NCORES):
        j, i = c % FSPLIT, c // FSPLIT
        full[
            j * F_SHARD : (j + 1) * F_SHARD, i * T_SHARD : (i + 1) * T_SHARD
        ] = res.results[c]["out"]
    return np.ascontiguousarray(full.T).reshape(B, S, OUT)
